# revision 39
# baseline (speedup 1.0000x reference)
"""Causal self-attention (RoPE, 16 heads, d=64, B=4, T=2048, C=1024) on 8 TRN2 cores.

Sharding: core g = (batch b = g//2, head-group hg = g%2 covering 8 heads).
Data-parallel over B, tensor-parallel over heads.  Each core computes the
partial out-projection (its 8 heads' contribution, no bias); the host sums
the two head-group partials per batch and adds b_out.

Per-core kernel (all matmul operands bf16, fp32 PSUM accumulation), emitted
as one interleaved stream per 512-wide t-window so the PE never starves:

  qkv segment I (dripped into attention window I-1 as PE gap-filler):
    q/k/v = xT.T @ Wqkv (xT pre-transposed on host as the stationary
    operand); RoPE on q,k in natural [t, d] layout on DVE (cos/sin muls
    against compact [T,64] tables via stride-0 broadcast APs); q,k
    HW-DMA-transposed into [d, t] layout (Sync queue carries ONLY
    transposes so the xbar never flips modes); v copied into [V | ones64]
    stationary tiles.  Input loads alternate between the GpSimd SWDGE
    queue and the Scalar HWDGE queue in consumption-order slices so the
    first qkv matmuls start ~2us in, chasing the loads.  Segment 0 (which
    runs before any attention) cycles its qkv PSUM tiles across the
    still-free attention banks for a 4-deep pipeline and sends its
    v-copies to the then-idle ACT engine, so the PE is not WAR-throttled
    behind DVE's rope reads while it chases the loads.

  attention window I, per head-pair:
    S^T[s,t] = k^T q with both heads packed in the PE array via
    tile_position row tiling (contraction dim is only 64); causal upper
    blocks skipped; exp on ACT (scale=1/8, padding mask as per-partition
    bias, no max subtraction -- logits are ~N(0,1)); the diagonal block's
    upper triangle is zeroed AFTER the exp by a cheap bf16 2x-mode DVE
    multiply with a 0/1 mask (no PSUM seeding matmuls);
    AV with [V | ones64] stationary and exp(S^T) streaming accumulates
    attn_out^T[d,t] on psT[0:64] and the denominator replicated on
    psT[64:128]; softmax division = both heads' denominators staged to
    one SBUF tile (custom DVE ops read matmul-written PSUM as garbage),
    one reciprocal_approx_fast over [128,512] (~5x faster than the
    iterative divide), and two DVE multiplies straight off PSUM into the
    out-projection's stationary layout (no attention-output transpose);
    out-projection units are deferred into the late ACT-bound windows
    where the PE otherwise idles behind the exp stream; "heater" matmuls
    keep the PE's HAM clock at 2.4 GHz when the drip runs dry.

  Output is stored bf16 (halves the store traffic); the host sums the two
  head-group partials in fp32 and adds b_out.
"""

import os
from contextlib import ExitStack

import numpy as np
import ml_dtypes

B, T, C = 4, 2048, 1024
H, D = 16, 64
HG = 8            # heads per core
NCORES = 8
TB = T // 128     # 16 t/s-blocks of 128
CBN = C // 128    # 8 contraction chunks
NP = HG // 2      # 4 head pairs
NI = T // 512     # 4 t-windows of 512
ROPE_BASE = 10000.0

_PROG = None
_LAST_RESULTS = None


def _build_program():
    import concourse.bass as bass
    import concourse.tile as tile
    from concourse import bacc, mybir

    f32 = mybir.dt.float32
    bf = mybir.dt.bfloat16
    EXP = mybir.ActivationFunctionType.Exp
    CPY = mybir.ActivationFunctionType.Copy

    nc = bacc.Bacc("TRN2", target_bir_lowering=False, debug=False)

    xT = nc.dram_tensor("xT", [C, T], bf, kind="ExternalInput").ap()
    wqkv = nc.dram_tensor("wqkv", [C, 3 * HG * D], bf, kind="ExternalInput").ap()
    wout = nc.dram_tensor("wout", [HG * D, C], bf, kind="ExternalInput").ap()
    cosT = nc.dram_tensor("cosT", [T, D], bf, kind="ExternalInput").ap()
    sinT = nc.dram_tensor("sinT", [T, D], bf, kind="ExternalInput").ap()
    padb = nc.dram_tensor("padb", [128, TB], f32, kind="ExternalInput").ap()
    tri = nc.dram_tensor("tri", [128, 128], bf, kind="ExternalInput").ap()
    outp = nc.dram_tensor("outp", [T, C], bf, kind="ExternalOutput").ap()

    with tile.TileContext(nc) as tc, ExitStack() as ctx:
        singles = ctx.enter_context(tc.tile_pool(name="singles", bufs=1))

        # ---- global SBUF tensors.  Input loads alternate between the
        # GpSimd SWDGE queue and the Scalar HWDGE queue (2x load bandwidth;
        # the Sync queue is reserved for DMA transposes so the xbar never
        # flips modes).  Order: (xt0,w0),(xt1,w1) so the first qkv matmuls
        # start ~5us in, then cos/sin (first RoPE), then the rest.
        xt_all = singles.tile([128, CBN, T], bf, name="xt_all", tag="xt_all")
        w_all = singles.tile([128, CBN, 3 * HG * D], bf, name="w_all",
                             tag="w_all")
        wo_all = singles.tile([128, 4, C], bf, name="wo_all", tag="wo_all")
        cos_sb = singles.tile([128, TB, D], bf, name="cos_sb", tag="cos_sb")
        sin_sb = singles.tile([128, TB, D], bf, name="sin_sb", tag="sin_sb")
        padb_sb = singles.tile([128, TB], f32, name="padb_sb", tag="padb_sb")
        tri_sb = singles.tile([128, 128], bf, name="tri_sb", tag="tri_sb")

        # Load order is pipelined against segment 0's consumption: the
        # first qkv matmul only needs xt[cb][:, 0:512] (covers tb 0-3) and
        # the q-columns of w[cb], so those 256KB slices go first and the PE
        # starts ~1.5us in, chasing the loads.  k/v weight columns, rope
        # tables, the t>=512 remainder of xT, and wout follow in
        # consumption order.
        g, s = nc.gpsimd, nc.scalar
        xTr = xT.rearrange("(cb p) t -> p cb t", p=128)
        wr = wqkv.rearrange("(cb p) c -> p cb c", p=128)
        wor = wout.rearrange("(c p) n -> p c n", p=128)
        H2 = CBN // 2
        # Early loads stay per-cb fine-grained so the first q matmuls wait
        # on 32-128KB slices, not merged monoliths (range-tracked deps);
        # late bulk loads are merged into few big DMAs.  Big transfers are
        # kept OFF the Scalar queue once attention nears: an HWDGE issue
        # that hits ring backpressure blocks the ACT engine (observed
        # 13.4us!) and with it the whole exp stream.
        loads = []
        for cb in range(CBN):
            a = (g, s) if cb % 2 == 0 else (s, g)
            loads.append((a[0], xt_all[:, cb, 0:128], xTr[:, cb, 0:128]))
            loads.append((a[1], w_all[:, cb, 0:512], wr[:, cb, 0:512]))
        # xt[:,128:512] immediately after the q weights: with q-first unit
        # ordering, q(tb1..3) consume it ~12us in (it previously sat behind
        # wk+wv and stalled the PE 8.8us); both halves on GpSimd so the
        # ACT engine's queue stays clear for the rope staging copies
        loads += [
            (g, xt_all[:, 0:H2, 128:512], xTr[:, 0:H2, 128:512]),
            (g, xt_all[:, H2:CBN, 128:512], xTr[:, H2:CBN, 128:512]),
            (g, cos_sb, cosT.rearrange("(tb p) d -> p tb d", p=128)),
            (s, sin_sb, sinT.rearrange("(tb p) d -> p tb d", p=128)),
        ]
        for cb in range(CBN):
            loads.append((g if cb % 2 else s,
                          w_all[:, cb, 512:1024], wr[:, cb, 512:1024]))
        loads += [
            # wv-h2/xtC1-h2 on GpSimd: every scalar-queue issue precedes the
            # rope staging copies in ACT program order, and these two were
            # the big ones delaying the copy chain (ACT idle-until-23us)
            (g, w_all[:, 0:H2, 1024:1536], wr[:, 0:H2, 1024:1536]),
            (g, w_all[:, H2:CBN, 1024:1536], wr[:, H2:CBN, 1024:1536]),
            (s, padb_sb, padb),
            (s, tri_sb, tri),
            (g, xt_all[:, 0:H2, 512:1024], xTr[:, 0:H2, 512:1024]),
            (g, xt_all[:, H2:CBN, 512:1024], xTr[:, H2:CBN, 512:1024]),
            (g, xt_all[:, 0:H2, 1024:T], xTr[:, 0:H2, 1024:T]),
            (g, xt_all[:, H2:CBN, 1024:T], xTr[:, H2:CBN, 1024:T]),
            (g, wo_all[:, 0:2, :], wor[:, 0:2, :]),
            (g, wo_all[:, 2:4, :], wor[:, 2:4, :]),
        ]
        for q, dst, src in loads:
            q.dma_start(out=dst, in_=src)

        # q^T/k^T: [within-pair col (head-lo d / head-hi d), s-block, pair, t]
        qT_all = singles.tile([128, TB, NP, 128], bf, name="qT_all", tag="qT_all")
        kT_all = singles.tile([128, TB, NP, 128], bf, name="kT_all", tag="kT_all")
        # v with 64 ones columns per head: the AV matmul then emits the
        # softmax denominator replicated on 64 partitions (rows 64-127),
        # so the reciprocal runs wide instead of a 1-partition crawl
        vones = singles.tile([128, TB, HG, 128], bf, name="vones", tag="vones")
        # only window 0's ones blocks up front (the full-tensor memset was
        # 6.9us of DVE squarely on the startup critical chain); each later
        # segment memsets its own blocks as part of the drip
        nc.vector.memset(vones[:, 0:4, :, D:128], 1.0)

        # qkv + attention interleaved per 512-wide t-window so the PE stream
        # stays dense (HAM stays at 2.4 GHz): attention for window I only
        # needs q/k/v blocks 0..4I+3, which segment I of the qkv loop topped
        # off.  One shared PSUM pool: qkv 2 + sAB 2x2 + psT 2 = 8 banks.
        with tc.tile_pool(name="psum", bufs=2, space="PSUM") as psum, \
             tc.tile_pool(name="rope", bufs=6) as rope_pool, \
             tc.tile_pool(name="qknat", bufs=8) as qk_pool, \
             tc.tile_pool(name="exps", bufs=3) as exp_pool, \
             tc.tile_pool(name="attnT", bufs=4) as aT_pool, \
             tc.tile_pool(name="recips", bufs=2) as rc_pool, \
             tc.tile_pool(name="outsb", bufs=4) as out_pool:
            def qkv_segment(I):
                """Generator emitting segment I's qkv matmuls in half-tensor
                chunks (yield points), so the caller can drip them into the
                attention loop as PE gap-filler while ACT grinds exps."""
                if I > 0:
                    # Pool is idle mid-kernel (its load-issue burst is over);
                    # keep DVE free for rope/normalize
                    nc.gpsimd.memset(vones[:, 4 * I:4 * I + 4, :, D:128], 1.0)
                # Segment 0 runs before the attention windows, so the sAB/
                # psT PSUM banks are still free: cycle its qkv tiles across
                # them for a 4-deep pipeline (the 2-buffer qkv rotation
                # otherwise WAR-stalls the PE behind DVE's rope reads of the
                # previous-but-one unit).  Its v-copies go to the then-idle
                # ACT engine for the same reason.
                tags = ("qkv", "avA", "qkv", "avB") if I == 0 else ("qkv",)
                u = 0
                # q units first: window I+1's FIRST S needs ALL four new q
                # transposes, while the new k blocks (and vones) are only
                # touched ~23% into that window.  With q last (old per-tb
                # order) the final q transpose + its multi-us semaphore lag
                # landed right at the window boundary and stalled the S.
                units = [(tb, "q", 0) for tb in range(4 * I, 4 * I + 4)]
                for tb in range(4 * I, 4 * I + 4):
                    units.append((tb, "k", 512))
                    units.append((tb, "v", 1024))
                for tb, which, base in units:
                    if True:
                        tag = tags[u % len(tags)]
                        u += 1
                        ps = psum.tile([128, HG, D], f32, name=f"ps{which}",
                                       tag=tag, bufs=(2 if tag == "qkv" else 1))
                        for cb in range(CBN):
                            nc.tensor.matmul(
                                ps, xt_all[:, cb, tb * 128:(tb + 1) * 128],
                                w_all[:, cb, base:base + 512],
                                start=(cb == 0), stop=(cb == CBN - 1))
                            if cb == 3:
                                yield
                        if which == "v":
                            if I == 0:
                                nc.scalar.activation(
                                    out=vones[:, tb, :, 0:D], in_=ps, func=CPY)
                            else:
                                nc.vector.tensor_copy(
                                    out=vones[:, tb, :, 0:D], in_=ps)
                            yield
                            continue
                        # rope: P_c = qkv*cosF, P_s = qkv*sinF (compact [T,64]
                        # tables broadcast over the 8 heads; both halves of
                        # each head carry the same table value), then
                        # lo = P_c.lo - P_s.hi ; hi = P_s.lo + P_c.hi
                        cosb = cos_sb[:, tb].unsqueeze(1).broadcast_to((128, HG, D))
                        sinb = sin_sb[:, tb].unsqueeze(1).broadcast_to((128, HG, D))
                        # bf16 intermediates: the sub/add below then run in
                        # DVE 2x mode (all-SBUF 2-byte packed operands)
                        pc = rope_pool.tile([128, HG, D], bf, name="pc", tag="rt")
                        psn = rope_pool.tile([128, HG, D], bf, name="psn", tag="rt")
                        if I == 0:
                            # segment 0's window-0 start is rope-THROUGHPUT
                            # bound: stage PSUM->SBUF bf16 via the idle ACT
                            # engine so the muls also hit DVE 2x mode
                            # (PSUM-sourced ops never do), halving the rope
                            # chain that gates the first S matmuls
                            qn = rope_pool.tile([128, HG, D], bf,
                                                name="qn", tag="rt")
                            nc.scalar.activation(out=qn, in_=ps, func=CPY)
                            rsrc = qn
                        else:
                            rsrc = ps
                        nc.vector.tensor_mul(pc, rsrc, cosb)
                        nc.vector.tensor_mul(psn, rsrc, sinb)
                        ro = qk_pool.tile([128, HG, D], bf, name="ro", tag="ro")
                        nc.vector.tensor_sub(
                            ro[:, :, 0:32], pc[:, :, 0:32], psn[:, :, 32:64])
                        nc.vector.tensor_add(
                            ro[:, :, 32:64], psn[:, :, 0:32], pc[:, :, 32:64])
                        dst = qT_all if which == "q" else kT_all
                        nc.sync.dma_start_transpose(out=dst[:, tb, :, :], in_=ro)
                        yield

            pending_out = []
            # out-proj PSUM tag: sAB normally; in window 3 the qkv-tag banks
            # are free (no drip), and using them avoids the WAR against the
            # exp stream that the sAB tag carries
            out_tag = ["sAB"]
            store_q = [nc.gpsimd]
            flushing = [False]

            # window-3 out-proj split: pairs 0-2 pre-accumulated into spill
            # tiles during pair 3's attention; finals add the pair-3 chunk
            spills = [[singles.tile([128, 512], bf, name=f"sp{il}{n}",
                                    tag=f"sp{il}{n}") for n in range(2)]
                      for il in range(4)]

            def make_partial(aT, il, n, sp):
                def emit():
                    psp = psum.tile([128, 512], f32, name="psp",
                                    tag="qkv", bufs=2)
                    for c in range(3):
                        nc.tensor.matmul(
                            psp, aT[:, c, il * 128:(il + 1) * 128],
                            wo_all[:, c, n * 512:(n + 1) * 512],
                            start=(c == 0), stop=(c == 2))
                    nc.vector.tensor_copy(out=sp, in_=psp)
                return emit

            def make_final(aT, il, n, sp):
                def emit():
                    pso = psum.tile([128, 512], f32, name="pso",
                                    tag="qkv", bufs=2)
                    nc.tensor.matmul(
                        pso, aT[:, 3, il * 128:(il + 1) * 128],
                        wo_all[:, 3, n * 512:(n + 1) * 512],
                        start=True, stop=True)
                    osb = out_pool.tile([128, 512], bf, name="osb", tag="osb")
                    nc.vector.tensor_add(osb, pso, sp)
                    store_q[0].dma_start(
                        out=outp[(12 + il) * 128:(13 + il) * 128,
                                 n * 512:(n + 1) * 512], in_=osb)
                    # alternate queues so the last burst drains twice as fast
                    store_q[0] = (nc.scalar if store_q[0] is nc.gpsimd
                                  else nc.gpsimd)
                return emit
            for I in range(NI):
                if I == 0:
                    for _ in qkv_segment(0):
                        pass
                nxt = qkv_segment(I + 1) if I + 1 < NI else None
                if nxt is None:
                    out_tag[0] = "qkv"
                n_chunks = 4 * 3 * 3  # yield points per segment
                n_iters = 4 * (4 * I + 4)
                emitted = it = 0
                # out-projection pop budget per window: defer most of it into
                # the late, ACT-bound windows where the PE otherwise idles
                # behind the exp stream (window 3 has no qkv drip left, so
                # it gets the bulk of the deferred units)
                pops_left = (0, 4, 6, 10 ** 9)[I]

                def drip(force=0):
                    nonlocal emitted
                    got = 0
                    if nxt is None:
                        return got
                    # with q units dripped first, their transposes are done
                    # by mid-window, so the drip can pace deeper into the
                    # window (~90%) and leave fewer dry late iterations
                    due = (10 * it * n_chunks) // (9 * n_iters)
                    while emitted < due or got < force:
                        if next(nxt, "done") == "done":
                            break
                        emitted += 1
                        got += 1
                    return got

                # ---- attention window I ----
                aT_I = aT_pool.tile([128, NP, 512], bf, name="aT_I", tag="aT_I")
                for p in range(NP):
                    psTA = psum.tile([128, 512], f32, name="psTA", tag="avA", bufs=1)
                    psTB = psum.tile([128, 512], f32, name="psTB", tag="avB", bufs=1)

                    def emit_av(j, eAB):
                        off = max(j - 4 * I, 0) * 128
                        for h2, psT in ((0, psTA), (1, psTB)):
                            nc.tensor.matmul(
                                psT[:, off:512],
                                vones[:, j, 2 * p + h2, :],
                                eAB[:, h2, off:512],
                                start=(j == 0), stop=(j == 4 * I + 3))

                    prev = None
                    dry = False
                    for j in range(4 * I + 4):
                        jl = j - 4 * I
                        off = max(jl, 0) * 128
                        sAB = psum.tile([128, 2, 512], f32, name="sAB", tag="sAB", bufs=2)
                        if dry:
                            # "heater" matmul: PE would otherwise sit ~50%
                            # idle behind ACT and HAM-downclock to 1.2 GHz;
                            # burn a throwaway matmul into the bank the next
                            # S matmul overwrites anyway
                            nc.tensor.matmul(
                                sAB[:, 0, :], xt_all[:, 0, 0:128], xt_all[:, 0, 0:512],
                                start=True, stop=True, skip_group_check=True)
                        HALVES = ((0, slice(0, 64)), (1, slice(64, 128)))
                        if jl >= 0:
                            # one matmul per head-half covering the diagonal
                            # block AND the queries after it (the diag upper
                            # triangle is cleaned post-exp by the tri mask):
                            # halves the instruction + LDWEIGHTS count here
                            for h2, rows in HALVES:
                                nc.tensor.matmul(
                                    sAB[:, h2, off:512],
                                    kT_all[rows, j, p, :],
                                    qT_all[rows, 4 * I + jl:4 * I + 4, p, :],
                                    start=True, stop=True,
                                    tile_position=(h2 * 64, 0),
                                    skip_group_check=True)
                        else:
                            for h2, rows in HALVES:
                                nc.tensor.matmul(
                                    sAB[:, h2, :],
                                    kT_all[rows, j, p, :],
                                    qT_all[rows, 4 * I:4 * I + 4, p, :],
                                    start=True, stop=True,
                                    tile_position=(h2 * 64, 0))
                        eAB = exp_pool.tile([128, 2, 512], bf, name="eAB", tag="eAB")
                        bias = padb_sb[:, j:j + 1]
                        nc.scalar.activation(
                            out=eAB[:, :, off:512], in_=sAB[:, :, off:512],
                            func=EXP, bias=bias, scale=0.125)
                        if jl >= 0:
                            # zero the diagonal block's upper triangle (keys
                            # after the query) post-exp: bf16 2x-mode DVE
                            # multiply with the 0/1 mask, replacing the PSUM
                            # -1e30 seeding matmuls
                            trib = tri_sb.unsqueeze(1).broadcast_to((128, 2, 128))
                            nc.vector.tensor_mul(
                                eAB[:, :, off:off + 128],
                                eAB[:, :, off:off + 128], trib)
                        if prev is not None:
                            emit_av(*prev)
                        prev = (j, eAB)
                        it += 1
                        did_out = False
                        # in window 3's last pair, spread the pops (the 8
                        # spill partials) across the 16 iterations instead of
                        # draining them all immediately
                        pstride = 2 if (I == NI - 1 and p == NP - 1) else 1
                        if pending_out and pops_left > 0 and it % pstride == 0:
                            pending_out.pop(0)()
                            pops_left -= 1
                            did_out = True
                        dry = drip() == 0 and not did_out and (it % 2 == 0)
                    emit_av(*prev)

                    # pair-boundary fill: the next pair's first S WAR-stalls
                    # on this pair's last exp (sAB buffer rotation), and its
                    # first AV on this pair's normalize reads (psT).  Keep
                    # the PE fed across that window with an out-proj pop or
                    # a couple of qkv drip chunks.
                    if pending_out and pops_left > 0:
                        pending_out.pop(0)()
                        pops_left -= 1
                    else:
                        drip(force=2)

                    # softmax normalization: stage both heads' replicated
                    # denominators (psT[64:128]) into one SBUF tile, one
                    # reciprocal_approx_fast over [128,512] (custom DVE op,
                    # ~5x faster than the iterative divide -- but it reads
                    # matmul-written PSUM as garbage, hence the SBUF staging
                    # copies), then multiply the numerators straight off
                    # PSUM into the out-projection's stationary layout.
                    # the staging copies shift partitions (64:128 -> 0:64),
                    # which only DVE can do (GPSIMD can't even touch PSUM)
                    cpd = rc_pool.tile([128, 512], f32, name="cpd", tag="cpd")
                    nc.vector.tensor_copy(out=cpd[0:64, :], in_=psTA[D:128, :])
                    nc.vector.tensor_copy(out=cpd[64:128, :], in_=psTB[D:128, :])
                    rc = rc_pool.tile([128, 512], f32, name="rc", tag="rc")
                    nc.vector.reciprocal_approx_fast(rc, cpd)
                    nc.vector.tensor_mul(
                        aT_I[0:64, p, :], psTA[0:D, :], rc[0:64, :])
                    nc.vector.tensor_mul(
                        aT_I[64:128, p, :], psTB[0:D, :], rc[64:128, :])

                    if I == NI - 1 and p == NP - 2:
                        # window-3 tail prep: pre-accumulate each out unit's
                        # pairs-0..2 contribution during pair 3's attention
                        # (fills the PE behind pair 3's exp stream) and spill
                        # to SBUF; the post-attention tail then needs only
                        # the small pair-3 matmul + one DVE add per unit
                        for il in range(4):
                            for n in range(2):
                                pending_out.append(
                                    make_partial(aT_I, il, n, spills[il][n]))

                # out-projection units are deferred into the next window's
                # loop as more PE gap-filler
                def make_out_unit(aT, i, il, n):
                    def emit():
                        pso = psum.tile([128, 512], f32, name="pso",
                                        tag=out_tag[0], bufs=2)
                        for c in range(4):
                            nc.tensor.matmul(
                                pso,
                                aT[:, c, il * 128:(il + 1) * 128],
                                wo_all[:, c, n * 512:(n + 1) * 512],
                                start=(c == 0), stop=(c == 3))
                        osb = out_pool.tile([128, 512], bf, name="osb", tag="osb")
                        nc.vector.tensor_copy(out=osb, in_=pso)
                        store_q[0].dma_start(
                            out=outp[i * 128:(i + 1) * 128, n * 512:(n + 1) * 512],
                            in_=osb)
                        if store_q[0] is not nc.gpsimd:
                            store_q[0] = nc.gpsimd
                        elif flushing[0]:
                            # alternate queues during the final flush so the
                            # last burst of stores drains twice as fast
                            store_q[0] = nc.scalar
                    return emit
                if I < NI - 1:
                    for il in range(4):
                        for n in range(2):
                            pending_out.append(
                                make_out_unit(aT_I, 4 * I + il, il, n))
                    for _ in nxt:
                        pass
                else:
                    # drain any unpopped units/partials, then the finals
                    for f in pending_out:
                        f()
                    pending_out.clear()
                    flushing[0] = True
                    for il in range(4):
                        for n in range(2):
                            make_final(aT_I, il, n, spills[il][n])()

    nc.compile()
    return nc


def _get_program():
    global _PROG
    if _PROG is None:
        _PROG = _build_program()
    return _PROG


def _rope_tables():
    bf16 = ml_dtypes.bfloat16
    inv = 1.0 / (ROPE_BASE ** (np.arange(0, D, 2, dtype=np.float64) / D))
    f = np.arange(T, dtype=np.float64)[:, None] * inv[None, :]  # [T, 32]
    c = np.cos(f)
    s = np.sin(f)
    # both 32-col halves carry the same table value
    cosT = np.concatenate([c, c], axis=1).astype(bf16)  # [T, 64]
    sinT = np.concatenate([s, s], axis=1).astype(bf16)
    return cosT, sinT


def kernel(x, attention_mask, W_qkv, W_out, b_out):
    global _LAST_RESULTS
    from concourse.bass_utils import run_bass_kernel_spmd

    nc = _get_program()
    bf16 = ml_dtypes.bfloat16
    x = np.asarray(x, dtype=np.float32)
    attention_mask = np.asarray(attention_mask)
    W_qkv = np.asarray(W_qkv, dtype=np.float32)
    W_out = np.asarray(W_out, dtype=np.float32)
    b_out = np.asarray(b_out, dtype=np.float32)

    cosT, sinT = _rope_tables()
    # causal mask for the diagonal block: keep key s <= query t
    tri = np.where(np.arange(128)[:, None] <= np.arange(128)[None, :], 1.0, 0.0)
    tri = tri.astype(bf16)

    in_maps = []
    for g in range(NCORES):
        b, hg = g // 2, g % 2
        sl = slice(hg * 512, hg * 512 + 512)
        wq = W_qkv[:, 0 * C:][:, sl]
        wk = W_qkv[:, 1 * C:2 * C][:, sl]
        wv = W_qkv[:, 2 * C:3 * C][:, sl]
        wqkv_g = np.ascontiguousarray(
            np.concatenate([wq, wk, wv], axis=1)).astype(bf16)
        xT_g = np.ascontiguousarray(x[b].T).astype(bf16)
        wout_g = np.ascontiguousarray(W_out[sl, :]).astype(bf16)
        padb_g = np.ascontiguousarray(
            np.where(attention_mask[b] != 0, 0.0, -1e30)
            .astype(np.float32).reshape(TB, 128).T)
        in_maps.append({
            "xT": xT_g, "wqkv": wqkv_g, "wout": wout_g,
            "cosT": cosT, "sinT": sinT, "padb": padb_g, "tri": tri,
        })

    res = run_bass_kernel_spmd(nc, in_maps, list(range(NCORES)))
    _LAST_RESULTS = res
    out = np.empty((B, T, C), dtype=np.float32)
    for b in range(B):
        out[b] = (res.results[2 * b]["outp"].astype(np.float32)
                  + res.results[2 * b + 1]["outp"].astype(np.float32) + b_out)
    return out

